# revision 44
# baseline (speedup 1.0000x reference)
"""Grouped Conv2D (32 groups of 8->8 ch, 3x3, SAME) on 8 trn2 NeuronCores.

Strategy:
  - Data-parallel over batch: 32 images / 8 cores = 4 images per core.
  - 2x2 PE-array tiling (4 concurrent 64x64 tiles): 2 column-groups each
    stream a different image's row-strip; the 2 row-tiles of a column
    hold block-diagonal weights for 8 groups (8 x [8ic x 8oc]).  This is
    the streaming-rate sweet spot: waves of 4 (LDWEIGHTS+MATMUL) pairs
    sustain one ~504-column wave per ~215 ns with weight loads fully
    hidden.  (4x4 tiling with 16 32x32 tiles was tried and is ~3x worse:
    with all rhs XBUSes busy, the 16 weight loads cannot overlap the
    streams, each wave serializes LDW->MM, and the resulting <70%
    MAC-array duty cycle keeps the HAM clock-gate at K=4/8 -- the whole
    kernel runs at 1.2 GHz instead of 2.4.)
  - Short PE warm-up (5 zero-weight waves) bridges engine-boot (~7 us)
    to first-input-arrival (~9 us) and starts the HAM clock ramp early.
  - Input DMA: per-chunk image-batched SBUF tiles so the SW-DGE path is
    4 large consolidated DMAs (each dma_start costs ~660 ns of engine
    setup time -- many small slices serialize).  Only the two images
    that gate the first batch get fast HW-DGE row-slices; the weight
    pack rides first on the sync ring.
  - Batch = strip s of an image PAIR, so image img's output always lands
    on PSUM partitions 64*(img%2)..+64.  Strips accumulate into a
    per-(chunk,pair) SBUF buffer [128, 2, 56, 56], which makes the
    output HBM DMA contiguous per channel (multi-KB descriptors instead
    of 896 B chunks), issued in 4 staggered waves so stores overlap
    compute and the kernel-ending store is a single 2-image strip.
  - Per batch: 9 taps x 4 tile-matmuls accumulate in PSUM (2 banks,
    4 batches in flight), then one engine copy (scalar/vector
    alternating) evacuates to bf16 SBUF.
  - bf16 in/out over HBM (host casts + pads), fp32 PSUM accumulate.
"""

import sys

import numpy as np

if "/opt/trn_rl_repo" not in sys.path:
    sys.path.insert(0, "/opt/trn_rl_repo")

import ml_dtypes

B, C, H, W = 32, 256, 56, 56
KK = 3
GROUPS = 32
CPG = 8  # in- and out-channels per group
N_CORES = 8
BPC = B // N_CORES  # images per core
NPAIR = BPC // 2  # image pairs per core
HP, WP = H + 2, W + 2  # padded image
NCHUNK = 2  # 256 channels = 2 x 128 partitions
NTAP = KK * KK
NCOL = 2  # column-groups (images per batch)
NROW = 2  # row-tiles per column (8 groups each)
GPT = 8  # groups per 64x64 tile
WU_FD = 504  # warm-up free dim (matches the 9-row strips)
WU_ROUNDS = 8  # warm-up bridges boot (~7.4us) to input-ready (~10.5us)
# ragged strips (row0, nrows): five 9-row strips (N=504 <= 512 fp32/bank)
# tapering to 8- and 3-row strips, so the kernel-ending evac+store chain
# is short and the second-to-last store overlaps the last batch.
STRIPS = [(0, 9), (9, 9), (18, 9), (27, 9), (36, 9), (45, 8), (53, 3)]
# strip si consumes padded input rows [row0, row0+nrows+2)
# fast-path (HW-DGE) row-slices for images 0/1: [0,11) gates the first
# batch, [11,20) the second
FAST_SUB = [(0, 11), (11, 20)]
# bulk (SW-DGE) row-slices for images 0/1, sized per strip deadline
BULK01 = [(20, 29), (29, 38), (38, 47), (47, 56), (56, 58)]
# coarse slices for everything else
SLICES = [(0, 20), (20, 47), (47, 58)]
# output waves: after strip `trig`, store output rows [r0, r1).  Grouped
# waves keep the scalar/sync engines' DMA-issue load (~0.65us per
# dma_start) low; the kernel-ending pair uses per-strip waves so its
# stores overlap its own compute and the tail stays short.
WAVES = [(0, 0, 9), (2, 9, 27), (4, 27, 45), (5, 45, 53), (6, 53, 56)]
WAVES_LAST = [(si, r0, r0 + nr) for si, (r0, nr) in enumerate(STRIPS)]


def _pack_weights(w: np.ndarray) -> np.ndarray:
    """[256, 8, 3, 3] fp32 -> [128 pc, 2 ck, 9 tap, 64] bf16.

    wpk[64r + 8j + ic, ck, 3*th+tw, 8j + oc] = w[128ck + 64r + 8j + oc, ic, th, tw]
    """
    wr = w.reshape(NCHUNK, NROW, GPT, CPG, CPG, KK, KK)  # ck, r, j, oc, ic, th, tw
    wpk = np.zeros((NROW, GPT, CPG, NCHUNK, NTAP, GPT, CPG), dtype=np.float32)
    for j in range(GPT):
        # [ck, r, oc, ic, th, tw] -> [r, ic, ck, (th tw), oc]
        blk = wr[:, :, j].transpose(1, 3, 0, 4, 5, 2).reshape(NROW, CPG, NCHUNK, NTAP, CPG)
        wpk[:, j, :, :, :, j, :] = blk
    return wpk.reshape(128, NCHUNK, NTAP, 64).astype(ml_dtypes.bfloat16)


def _build_bass():
    import concourse.tile as tile
    from concourse import bacc, mybir

    nc = bacc.Bacc()
    xs = nc.dram_tensor(
        "xs", [BPC, C, HP, WP], mybir.dt.bfloat16, kind="ExternalInput"
    )
    wpk = nc.dram_tensor(
        "wpk", [128, NCHUNK, NTAP, 64], mybir.dt.bfloat16, kind="ExternalInput"
    )
    out = nc.dram_tensor(
        "out", [BPC, C, H, W], mybir.dt.bfloat16, kind="ExternalOutput"
    )

    with tile.TileContext(nc) as tc:
        with (
            tc.tile_pool(name="singles", bufs=1) as singles,
            tc.tile_pool(name="xpad_pool", bufs=8) as xpad_pool,
            tc.tile_pool(name="obuf_pool", bufs=2) as obuf_pool,
            tc.tile_pool(name="psum_pool", bufs=4, space="PSUM") as psum_pool,
        ):
            # warm-up on a memset scratch tile: needs no input data, starts
            # right after engine boot, covers the HAM clock ramp while the
            # first input slices and weights stream in.
            wu_src = singles.tile([128, 512], mybir.dt.bfloat16)
            nc.vector.memset(wu_src[:], 0.0)
            wu = psum_pool.tile([128, NROW, 512], mybir.dt.float32, name="ps")
            for _ in range(WU_ROUNDS):
                for cg in range(NCOL):
                    for r in range(NROW):
                        nc.tensor.matmul(
                            wu[64 * cg : 64 * cg + 64, r, :WU_FD],
                            lhsT=wu_src[64 * r : 64 * r + 64, :64],
                            rhs=wu_src[64 * r : 64 * r + 64, :WU_FD],
                            start=True,
                            stop=True,
                            tile_position=(64 * r, 64 * cg),
                        )

            # per-(chunk, image) input tiles, streamed as 3 row-slices in
            # consumption order.  The startup DMA rate is HBM-pair-limited
            # (all 8 cores load at once), so the first batch's gate is kept
            # small: chunk-0 weights (147 KB) + images 0/1 rows [0,18)
            # (127 KB each) ride the two HW-DGE rings; everything else goes
            # on the SW-DGE queue.
            xpads = {}
            for ck in range(NCHUNK):
                for img in range(BPC):
                    xpads[(ck, img)] = xpad_pool.tile(
                        [128, HP, WP], mybir.dt.bfloat16, name="xpad"
                    )
            w_sb = singles.tile([128, NCHUNK, NTAP, 64], mybir.dt.bfloat16)
            # critical path, balanced across the two HW-DGE rings: the
            # first batch's gate is wpk taps 0-4 + images 0/1 rows [0,11).
            # (Finer splitting was tried and is a net loss: each extra
            # dma_start costs ~0.65us of engine issue time + ~0.9us of
            # completion-semaphore latency.)
            nc.sync.dma_start(out=w_sb[:, 0, 0:5], in_=wpk[:, 0, 0:5])
            nc.scalar.dma_start(out=w_sb[:, 0, 5:9], in_=wpk[:, 0, 5:9])
            for lo, hi in FAST_SUB:
                nc.sync.dma_start(
                    out=xpads[(0, 0)][:, lo:hi, :], in_=xs[0, 0:128, lo:hi, :]
                )
                nc.scalar.dma_start(
                    out=xpads[(0, 1)][:, lo:hi, :], in_=xs[1, 0:128, lo:hi, :]
                )
            # Stall the SW-DGE queue ~3 us with a dummy memset so the
            # critical HW-DGE loads above get the (HBM-pair-contended)
            # startup bandwidth to themselves; the bulk loads still finish
            # well ahead of their consumers.
            gp_delay = singles.tile([128, 3600], mybir.dt.bfloat16)
            nc.gpsimd.memset(gp_delay[:], 0.0)
            for lo, hi in BULK01:
                for img in range(2):
                    nc.gpsimd.dma_start(
                        out=xpads[(0, img)][:, lo:hi, :],
                        in_=xs[img, 0:128, lo:hi, :],
                    )
            for img in range(2, BPC):
                lo, hi = SLICES[0]
                nc.gpsimd.dma_start(
                    out=xpads[(0, img)][:, lo:hi, :],
                    in_=xs[img, 0:128, lo:hi, :],
                )
            # chunk-1 weights are tiny and gate the chunk transition; load
            # them well ahead of their just-in-time position.
            nc.gpsimd.dma_start(out=w_sb[:, 1], in_=wpk[:, 1])
            for lo, hi in SLICES[1:]:
                for img in range(2, BPC):
                    nc.gpsimd.dma_start(
                        out=xpads[(0, img)][:, lo:hi, :],
                        in_=xs[img, 0:128, lo:hi, :],
                    )
            for lo, hi in SLICES:
                for img in range(BPC):
                    nc.gpsimd.dma_start(
                        out=xpads[(1, img)][:, lo:hi, :],
                        in_=xs[img, 128:256, lo:hi, :],
                    )

            # 28 batches = 2 chunks x 2 image-pairs x 7 strips; each batch
            # = strip s of both images of the pair (image = column-group).
            n_batch = 0
            for ck in range(NCHUNK):
                for p in range(NPAIR):
                    imgs = (2 * p, 2 * p + 1)
                    obuf = obuf_pool.tile(
                        [128, NROW, H, W], mybir.dt.bfloat16, name="obuf"
                    )
                    for si, (row0, nr) in enumerate(STRIPS):
                        fd = nr * W
                        ps = psum_pool.tile(
                            [128, NROW, 512], mybir.dt.float32, name="ps"
                        )
                        for t in range(NTAP):
                            th, tw = divmod(t, KK)
                            for cg in range(NCOL):
                                for r in range(NROW):
                                    nc.tensor.matmul(
                                        ps[64 * cg : 64 * cg + 64, r, :fd],
                                        lhsT=w_sb[64 * r : 64 * r + 64, ck, t, :],
                                        rhs=xpads[(ck, imgs[cg])][
                                            64 * r : 64 * r + 64,
                                            row0 + th : row0 + th + nr,
                                            tw : tw + W,
                                        ],
                                        start=(t == 0),
                                        stop=(t == NTAP - 1),
                                        tile_position=(64 * r, 64 * cg),
                                    )
                        # evac on scalar/vector alternating; the
                        # kernel-ending batch splits across both engines so
                        # the final chain is short.
                        last = (
                            ck == NCHUNK - 1
                            and p == NPAIR - 1
                            and si == len(STRIPS) - 1
                        )
                        rows = slice(row0, row0 + nr)
                        if last:
                            nc.scalar.copy(out=obuf[:, 0, rows, :], in_=ps[:, 0, :fd])
                            nc.vector.tensor_copy(
                                out=obuf[:, 1, rows, :], in_=ps[:, 1, :fd]
                            )
                        elif n_batch % 2 == 0:
                            nc.scalar.copy(out=obuf[:, :, rows, :], in_=ps[:, :, :fd])
                        else:
                            nc.vector.tensor_copy(
                                out=obuf[:, :, rows, :], in_=ps[:, :, :fd]
                            )
                        n_batch += 1
                        last_pair = ck == NCHUNK - 1 and p == NPAIR - 1
                        for trig, r0, r1 in (WAVES_LAST if last_pair else WAVES):
                            if si != trig:
                                continue
                            if last:
                                # kernel-ending wave: one DMA per (image,
                                # row-tile) so each fires as soon as its
                                # evac half lands, spread over both rings.
                                for cg in range(NCOL):
                                    img = imgs[cg]
                                    for r in range(NROW):
                                        dst = out[
                                            img,
                                            ck * 128 + 64 * r : ck * 128 + 64 * r + 64,
                                            r0:r1,
                                            :,
                                        ]
                                        src = obuf[
                                            64 * cg : 64 * cg + 64, r, r0:r1, :
                                        ]
                                        eng = nc.scalar if r == 0 else nc.sync
                                        eng.dma_start(out=dst, in_=src)
                                continue
                            for cg in range(NCOL):
                                img = imgs[cg]
                                dst = out[
                                    img,
                                    ck * 128 : (ck + 1) * 128,
                                    r0:r1,
                                    :,
                                ].rearrange("(r p) h w -> p r h w", r=NROW)
                                src = obuf[64 * cg : 64 * cg + 64, :, r0:r1, :]
                                eng = nc.scalar if cg == 0 else nc.sync
                                eng.dma_start(out=dst, in_=src)
    nc.finalize()
    return nc


_CACHE = {}


def kernel(x, w, trace=False):
    from concourse.bass_utils import run_bass_kernel_spmd

    x = np.asarray(x)
    w = np.ascontiguousarray(np.asarray(w), dtype=np.float32)

    if "nc" not in _CACHE:
        _CACHE["nc"] = _build_bass()
    nc = _CACHE["nc"]

    xbf = np.zeros((B, C, HP, WP), dtype=ml_dtypes.bfloat16)
    xbf[:, :, 1 : H + 1, 1 : W + 1] = x.astype(ml_dtypes.bfloat16)
    wpk = _pack_weights(w)
    in_maps = [
        {"xs": np.ascontiguousarray(xbf[i * BPC : (i + 1) * BPC]), "wpk": wpk}
        for i in range(N_CORES)
    ]
    res = run_bass_kernel_spmd(
        nc, in_maps, core_ids=list(range(N_CORES)), trace=trace
    )
    outs = np.concatenate([res.results[i]["out"] for i in range(N_CORES)], axis=0)
    if trace:
        kernel.last_result = res
    return outs.astype(np.float32)
